# revision 113
# baseline (speedup 1.0000x reference)
# Multi-head attention (N=2, S=2048, E=2048, H=16, Dk=128) on 8 NeuronCores.
#
# Sharding: 2 batches x 16 heads = 32 (n,h) pairs -> core c owns batch c//4,
# heads (c%4)*4 .. +4. The reference reshapes (N,H,S,Dk)->(N,S,H*Dk) without
# a head transpose, so rows [h*128,(h+1)*128) of the pre-projection matrix X
# (and hence of the final output) depend on head h only: each core computes
# 512 disjoint output rows and the host concatenates. No collectives.
#
# v7 (over v6): the softmax add-tree is split -- first half in-place across
# attention slots 4-6 as attnV releases those expT rows, second half right
# after the chunk evict -- so Vector's in-order queue never parks an evict
# behind a tail reciprocal; the tail's normalize-mul is emitted after the
# evict for the same reason. V-phase x tiles get a dedicated prefetch ring
# (vin) and wpool bufs=3 so wv's DMAs are not gated on Q-proj's last
# LDWEIGHTS; weight/x descriptors are spread across scalar/gpsimd/sync by
# phase. The last Q/K s-chunk's evicts alternate Scalar/Vector because the
# freed pa PSUM banks become the V/score rings. bo is folded in on the host
# (per-column additive constant), removing its 2.2us gpsimd broadcast; the
# drain reserves two head-2 chunks whose matmuls cover the final
# denominator chain, and the last O-proj chunk computes + evicts in column
# halves so the output DMA overlaps its matmuls.
import numpy as np

D_MODEL = 2048
NHEAD = 16
DK = 128
N_BATCH = 2
SEQ = 2048
N_CORES = 8
HEADS_PER_CORE = 4


class Cfg:
    def __init__(self, S=SEQ, E=D_MODEL, NH=HEADS_PER_CORE, CH=512):
        assert S % 128 == 0 and E % 128 == 0
        self.S = S          # sequence length
        self.E = E          # model dim (contraction for projections)
        self.NH = NH        # heads per core
        self.CH = CH        # s-chunk width for attention phase
        self.NK = E // 128  # contraction tiles for projections / O-proj
        self.NK2 = self.NK // 2  # k-pair tiles (DMA granularity)
        self.NT = S // 128  # t tiles (attention contraction)
        self.HDc = NH * DK  # head dims per core
        self.RPH = (S * DK) // E  # output rows per head (=128 at full size)
        assert self.RPH == 128, "O-proj layout assumes 128 rows per head"
        self.NCH = S // CH  # number of s-chunks
        assert S % CH == 0 and CH == 512
        self.PCH = 512      # projection / O-proj free-dim chunk
        self.NPC = S // self.PCH   # projection s-chunks
        self.NOC = E // self.PCH   # O-proj output chunks


def build_program(cfg: Cfg):
    import concourse.bass as bass
    import concourse.tile as tile
    from concourse import bacc, mybir
    from contextlib import ExitStack

    fp32 = mybir.dt.float32
    fp16 = mybir.dt.float16
    AF = mybir.ActivationFunctionType

    S, E, NH, CH = cfg.S, cfg.E, cfg.NH, cfg.CH
    NK, NK2, NT, HDc = cfg.NK, cfg.NK2, cfg.NT, cfg.HDc
    PCH, NPC, NOC, NCH = cfg.PCH, cfg.NPC, cfg.NOC, cfg.NCH
    inv_sqrt_dk = 1.0 / float(np.sqrt(DK))

    nc = bacc.Bacc("TRN2", target_bir_lowering=False, debug=False,
                   num_devices=N_CORES)

    # DRAM I/O (per-core values supplied via in_maps). x inputs are
    # host-pretiled fp16: [k_pair, s_chunk, partition, 2*512] so every DMA
    # reads contiguous 2KB partition rows.
    xq = nc.dram_tensor("xq", [NK2, NPC, 128, 1024], fp16,
                        kind="ExternalInput").ap()
    xk = nc.dram_tensor("xk", [NK2, NPC, 128, 1024], fp16,
                        kind="ExternalInput").ap()
    xv = nc.dram_tensor("xv", [NK2, NPC, 128, 1024], fp16,
                        kind="ExternalInput").ap()
    wq = nc.dram_tensor("wq", [NK2, 128, 2, HDc], fp16,
                        kind="ExternalInput").ap()
    wk = nc.dram_tensor("wk", [NK2, 128, 2, HDc], fp16,
                        kind="ExternalInput").ap()
    wv = nc.dram_tensor("wv", [NK2, 128, 2, HDc], fp16,
                        kind="ExternalInput").ap()
    wo = nc.dram_tensor("wo", [NOC, NK2, 128, 2, 512], fp16,
                        kind="ExternalInput").ap()
    bq = nc.dram_tensor("bq", [128, NH], fp32, kind="ExternalInput").ap()
    bk = nc.dram_tensor("bk", [128, NH], fp32, kind="ExternalInput").ap()
    bv = nc.dram_tensor("bv", [1, HDc], fp16, kind="ExternalInput").ap()
    ones_d = nc.dram_tensor("ones", [128, 1], fp16,
                            kind="ExternalInput").ap()
    out = nc.dram_tensor("out", [NH * 128, E], fp16, kind="ExternalOutput").ap()

    with tile.TileContext(nc) as tc, ExitStack() as ctx:
        consts = ctx.enter_context(tc.tile_pool(name="consts", bufs=1))
        ones_sb = consts.tile([128, 1], fp16)
        ones_col = ones_sb[:, :1]
        bq_sb = consts.tile([128, NH], fp32)
        bk_sb = consts.tile([128, NH], fp32)
        bv_sb = consts.tile([1, HDc], fp16)
        bv_bc = consts.tile([128, HDc], fp16)   # bv broadcast along t

        persist = ctx.enter_context(tc.tile_pool(name="persist", bufs=1))
        qc = persist.tile([128, NH, S], fp16)      # qT_c: [d, h, s]
        kc = persist.tile([128, NH, S], fp16)      # kT_c: [d, h, s]
        vc = persist.tile([128, NT, HDc], fp16)    # v_c: [t_p, t_t, h*128+d]
        oc = persist.tile([128, NH, S], fp16)      # attn out: [d, h, s]

        # SBUF pools that span phase A tail + phase B (pre-scored pairs).
        expp = ctx.enter_context(tc.tile_pool(name="expp", bufs=4))
        accp = ctx.enter_context(tc.tile_pool(name="accp", bufs=1))
        bsc = ctx.enter_context(tc.tile_pool(name="bsc", bufs=2))
        expT_of = {}

        # ============== Phase A: q/k/v projections ==============
        from contextlib import ExitStack as _ES
        a_ctx = _ES()
        # bufs=3: wq/wk/wv each get fresh slots, so wv's DMAs are not
        # gated on Q-proj's last LDWEIGHTS releasing wq's tiles (that
        # gating was a ~1.9us PE stall at the K->V transition)
        wpool = a_ctx.enter_context(tc.tile_pool(name="wpool", bufs=3))
        xin = a_ctx.enter_context(tc.tile_pool(name="xin", bufs=6))
        # dedicated prefetch ring for V x tiles: their DMAs hoist into the
        # K-projection window instead of waiting for xin slots, so the PE
        # has V matmuls available the moment K-proj ends.
        vin = a_ctx.enter_context(tc.tile_pool(name="vin", bufs=4))
        with tc.tile_pool(name="pa_psum", bufs=2, space="PSUM") as pa:

            def load_w(w_dram, engs, gate=None, first=False):
                # 4 sub-tiles of 2 k-pairs each so the first matmuls only
                # wait on the first pair of weight DMAs, not all eight.
                # `gate` writes one element into each destination tile so
                # the DMAs cannot be scheduled before the gate source is
                # produced (keeps them off the startup DMA window).
                # `first`: the leading k-pair is split k-granular so the
                # program's first LDWEIGHTS waits on a 128KB DMA, not 256.
                parts = []
                for i in range(4):
                    p = wpool.tile([128, 4, HDc], fp16, tag=f"w{i}",
                                   name=f"w{i}")
                    if gate is not None:
                        nc.gpsimd.tensor_copy(p[0:1, 0, 0:1], gate)
                    if first and i == 0:
                        engs[0].dma_start(p[:, 0:1, :], w_dram[0][:, 0:1, :])
                        engs[0].dma_start(p[:, 1:2, :], w_dram[0][:, 1:2, :])
                        engs[0].dma_start(p[:, 2:4, :], w_dram[1])
                    else:
                        for j in range(2):
                            engs[i].dma_start(p[:, 2 * j:2 * j + 2, :],
                                              w_dram[2 * i + j])
                    parts.append(p)
                return parts

            def proj_qk(w_sb, x_dram, bias_sb, dst, eng, eng2=None):
                # dst[:, m, s*] = W_c @ x^T  (hd x S), bias fused in evict
                for s in range(NPC):
                    ps = [pa.tile([128, PCH], fp32, tag=f"pa{m}",
                                  name=f"pa{m}") for m in range(NH)]
                    for k2 in range(NK2):
                        xtile = xin.tile([128, 1024], fp16, tag="xin")
                        if eng2 == "first" and s == 0 and k2 == 0:
                            # split the program's first x tile: matmuls
                            # k=0,1 only need the kk=0 half
                            eng.dma_start(xtile[:, 0:512],
                                          x_dram[0, 0][:, 0:512])
                            eng.dma_start(xtile[:, 512:1024],
                                          x_dram[0, 0][:, 512:1024])
                        else:
                            eng.dma_start(xtile[:], x_dram[k2, s])
                        for kk in range(2):
                            k = 2 * k2 + kk
                            xs = xtile[:, kk * 512:(kk + 1) * 512]
                            for m in range(NH):
                                nc.tensor.matmul(
                                    ps[m][:],
                                    w_sb[k // 4][:, k % 4,
                                                 m * 128:(m + 1) * 128],
                                    xs, start=(k == 0), stop=(k == NK - 1))
                    for m in range(NH):
                        # last s-chunk: alternate evict engines -- the
                        # freed pa banks become the V/score PSUM rings, so
                        # the serialized 2.4us Scalar evict train directly
                        # delays the next phase's first matmuls
                        if s == NPC - 1 and m % 2 == 1:
                            nc.vector.tensor_scalar_add(
                                dst[:, m, s * PCH:(s + 1) * PCH],
                                ps[m][:], bias_sb[:, m:m + 1])
                        else:
                            nc.scalar.activation(
                                dst[:, m, s * PCH:(s + 1) * PCH],
                                ps[m][:], AF.Identity,
                                bias=bias_sb[:, m:m + 1])

            # wq's later sub-tiles stream on gpsimd (idle at startup) so
            # the scalar descriptor queue only carries what gates the
            # first matmuls; wk streams on gpsimd behind a qc gate that
            # keeps it out of wq's startup DMA window; wv goes back on
            # scalar BEFORE K-proj is emitted, so its descriptors run
            # between the Q and K evict trains and the tiles are resident
            # well before the V matmuls start.
            def pa_ring_skip():
                # advance the pa PSUM ring one slot-set so K-proj's final
                # s-chunk lands on the other half of the ring: the banks
                # pav inherits are then free ~14us before V-proj starts
                # instead of being read by K's last evicts.
                for m in range(NH):
                    pa.tile([128, PCH], fp32, tag=f"pa{m}",
                            name=f"pa{m}_skip")

            wq_sb = load_w(wq, [nc.scalar, nc.scalar, nc.gpsimd, nc.gpsimd])
            nc.scalar.dma_start(ones_sb[:], ones_d)
            nc.scalar.dma_start(bq_sb[:], bq)
            nc.scalar.dma_start(bk_sb[:], bk)
            nc.scalar.dma_start(bv_sb[:], bv)
            proj_qk(wq_sb, xq, bq_sb, qc, nc.sync)
            # gate on Q's SECOND s-chunk evict (~39us): wk's 2MB stream
            # then stays out of the most contended stretch of the Q-phase
            # x/w traffic (8 cores all pulling weights at once); its real
            # deadline is K-proj start at ~67us.
            wk_sb = load_w(wk, [nc.gpsimd] * 4, gate=qc[0:1, 0, 512:513])
            wv_sb = load_w(wv, [nc.scalar] * 4)
            pa_ring_skip()
            proj_qk(wk_sb, xk, bk_sb, kc, nc.gpsimd)

            # bv broadcast tile (GpSimd partition-broadcast)
            nc.gpsimd.partition_broadcast(bv_bc[:], bv_sb[:])

        def emit_scores(cur):
            # scores + exp for one (h, c) pair; expT kept for the attnV
            # step that consumes it later.
            h, c = cur
            cs = slice(c * CH, (c + 1) * CH)
            expT = expp.tile([128, NT, CH], fp16, tag="expT",
                             name=f"expT_{h}_{c}")
            for tt2 in range(NT // 2):
                st = stp.tile([128, 2, 512], fp32, tag="st", name="st")
                for i in range(2):
                    tt = 2 * tt2 + i
                    nc.tensor.matmul(
                        st[:, i, :], kc[:, h, tt * 128:(tt + 1) * 128],
                        qc[:, h, cs], start=True, stop=True)
                nc.scalar.activation(expT[:, 2 * tt2:2 * tt2 + 2, :],
                                     st[:], AF.Exp, scale=inv_sqrt_dk)
            expT_of[cur] = expT

        atts = [(h, c) for h in range(NH) for c in range(NCH)]

        # pa closed: scores PSUM ring opens (spans pre-scores + phase B)
        stp = ctx.enter_context(
            tc.tile_pool(name="st_psum", bufs=2, space="PSUM"))

        with tc.tile_pool(name="pav_psum", bufs=1, space="PSUM") as pav:

            def proj_v_group(tc4):
                # 4 t-tiles of v: stationary = x tile slices, moving = w
                ps = [pav.tile([128, HDc], fp32, tag=f"pav{j}",
                               name=f"pav{j}") for j in range(4)]
                for k2 in range(NK2):
                    xtile = vin.tile([128, 1024], fp16, tag="vin")
                    # group 0 entirely on sync (gpsimd is still draining
                    # K descriptors when V-proj starts); mid groups' odd
                    # halves go on scalar, which is idle after K's evicts
                    if k2 % 2 == 0 or tc4 == 0:
                        eng = nc.sync
                    elif tc4 < 3:
                        eng = nc.scalar
                    else:
                        eng = nc.gpsimd
                    eng.dma_start(xtile[:], xv[k2, tc4])
                    for kk in range(2):
                        k = 2 * k2 + kk
                        for j in range(4):
                            xs = xtile[:, kk * 512 + j * 128:
                                       kk * 512 + (j + 1) * 128]
                            nc.tensor.matmul(
                                ps[j][:], xs, wv_sb[k // 4][:, k % 4, :],
                                start=(k == 0), stop=(k == NK - 1))
                for j in range(4):
                    nc.vector.tensor_add(vc[:, tc4 * 4 + j, :], ps[j][:],
                                         bv_bc[:])

            # Pre-score two pairs between the V groups (V first, so the
            # PE never waits on the K evicts): Scalar banks a 2-pair exp
            # lead during the v-projection; the lead absorbs Scalar's
            # per-step deficit for the whole attention phase.
            proj_v_group(0)
            emit_scores(atts[0])
            proj_v_group(1)
            proj_v_group(2)
            emit_scores(atts[1])
            for g in range(3, NT // 4):
                proj_v_group(g)

        a_ctx.close()

        # ============== Phase B: attention ==============
        wop = ctx.enter_context(tc.tile_pool(name="wop", bufs=4))
        wo_tiles = {}

        def load_wo(nn):
            # tiny vc-sourced copies gate each DMA so the scheduler cannot
            # hoist these dep-free loads into the phase-A DMA window; the
            # DMA issues go on sync (idle in early B) so they never delay
            # the per-tail partition ops on gpsimd.
            wo_t = wop.tile([128, NK, 512], fp16, tag="wo", name=f"wo{nn}")
            for k2 in range(NK2):
                nc.gpsimd.tensor_copy(wo_t[:, 2 * k2, 0:1],
                                      vc[:, NT - 1, HDc - 1:HDc])
                nc.sync.dma_start(wo_t[:, 2 * k2:2 * k2 + 2, :],
                                  wo[nn, k2])
            wo_tiles[nn] = wo_t

        with tc.tile_pool(name="op_psum", bufs=2, space="PSUM") as opp, \
             tc.tile_pool(name="pso_psum", bufs=1, space="PSUM") as psop, \
             tc.tile_pool(name="dn_psum", bufs=1, space="PSUM") as dnpp:

            def emit_tail_head(t):
                # denominator part 2a: ones-matmul column sum, fast
                # reciprocal, GpSimd partition-broadcast. The normalize
                # multiply is emitted separately AFTER the chunk evict so
                # a slow broadcast can never park the evict behind it on
                # Vector's in-order queue. (A GpSimd partition_all_reduce
                # variant was tried and reverted: its real 4-5us latency
                # is invisible to the static scheduler's cost model.)
                (ph, pc), acc, op = t
                dn = dnpp.tile([1, CH], fp32, tag="dn", name="dn")
                nc.tensor.matmul(dn[:], ones_col, acc[:],
                                 start=True, stop=True)
                rsc1 = bsc.tile([1, CH], fp32, tag="rsc1", name="rsc1")
                nc.vector.reciprocal_approx_fast(rsc1[:], dn[:])
                rsc = bsc.tile([128, CH], fp32, tag="rsc", name="rsc")
                nc.gpsimd.partition_broadcast(rsc[:], rsc1[:])
                return ((ph, pc), op, rsc)

            def emit_tail_mul(th):
                (ph, pc), op, rsc = th
                nc.vector.tensor_mul(oc[:, ph, pc * CH:(pc + 1) * CH],
                                     op[:], rsc[:])

            def emit_tail(t):
                emit_tail_mul(emit_tail_head(t))

            osb = ctx.enter_context(tc.tile_pool(name="osb", bufs=2))

            def start_chunk(h, nn, pool=None, tag="pso"):
                pool = pool or psop
                return {"h": h, "nn": nn, "k": 0,
                        "ps": pool.tile([128, PCH], fp32, tag=tag,
                                        name="pso"),
                        "ocv": oc[:, h, :].rearrange("p (j i) -> p i j",
                                                     i=NK),
                        "wo": wo_tiles[nn]}

            def chunk_mms(ch, n):
                for _ in range(n):
                    if ch is None or ch["k"] >= NK:
                        return
                    k = ch["k"]
                    nc.tensor.matmul(ch["ps"][:], ch["ocv"][:, k, :],
                                     ch["wo"][:, k, :],
                                     start=(k == 0), stop=(k == NK - 1))
                    ch["k"] += 1

            def finish_chunk(ch):
                # bo is folded in on the host after gather (exact: it is a
                # per-column additive constant), so the evict is a copy.
                if ch is None:
                    return
                chunk_mms(ch, NK - ch["k"])
                ns = slice(ch["nn"] * PCH, (ch["nn"] + 1) * PCH)
                ot = osb.tile([128, PCH], fp16, tag="osb")
                nc.vector.tensor_copy(ot[:], ch["ps"][:])
                # out DMAs issue on gpsimd: keeps that engine active every
                # step (its broadcasts run ~2x slower after idle spells)
                # and relieves sync, the busiest descriptor queue
                nc.gpsimd.dma_start(
                    out[ch["h"] * 128:(ch["h"] + 1) * 128, ns], ot[:])

            def finish_chunk_split(ch):
                # last chunk: compute + evict in two column halves so the
                # first half's bias-add and output DMA overlap the second
                # half's matmuls (shrinks the post-PE tail).
                ns0 = ch["nn"] * PCH
                ot = osb.tile([128, PCH], fp16, tag="osb")
                for half in range(2):
                    hs = slice(half * 256, (half + 1) * 256)
                    for k in range(NK):
                        nc.tensor.matmul(
                            ch["ps"][:, hs], ch["ocv"][:, k, :],
                            ch["wo"][:, k, hs],
                            start=(k == 0), stop=(k == NK - 1))
                    nc.vector.tensor_copy(ot[:, hs], ch["ps"][:, hs])
                    nc.sync.dma_start(
                        out[ch["h"] * 128:(ch["h"] + 1) * 128,
                            ns0 + half * 256:ns0 + (half + 1) * 256],
                        ot[:, hs])

            score_q = [(p, t) for p in range(2, 16) for t in range(8)]
            score_state = {"i": 0, "open": {}}

            def emit_score_slot():
                # one st-ring slot (2 tt) of the global score stream; the
                # per-step budget is lighter in the Scalar-bound early
                # steps and the deficit lands on steps 13-14 where Scalar
                # is otherwise idle
                if score_state["i"] >= len(score_q):
                    return
                p, tt2 = score_q[score_state["i"]]
                score_state["i"] += 1
                h, c = atts[p]
                if tt2 == 0:
                    score_state["open"][p] = expp.tile(
                        [128, NT, CH], fp16, tag="expT",
                        name=f"expT_{h}_{c}")
                expT = score_state["open"][p]
                cs = slice(c * CH, (c + 1) * CH)
                st = stp.tile([128, 2, 512], fp32, tag="st", name="st")
                for i in range(2):
                    tt = 2 * tt2 + i
                    nc.tensor.matmul(
                        st[:, i, :], kc[:, h, tt * 128:(tt + 1) * 128],
                        qc[:, h, cs], start=True, stop=True)
                nc.scalar.activation(expT[:, 2 * tt2:2 * tt2 + 2, :],
                                     st[:], AF.Exp, scale=inv_sqrt_dk)
                if tt2 == 7:
                    expT_of[atts[p]] = score_state["open"].pop(p)

            def att_step(av_pair, nslots, pending, ch, last=False):
                # Interleave scores(j+2) tt-pairs with attnV(j) tt-pairs
                # plus two O-proj matmuls per slot: the filler keeps each
                # st-ring slot interval above the Scalar exp latency so the
                # score stream is never exp-paced. The denominator add-tree
                # runs split: the first half in-place across slots 4-6 (as
                # the attnV matmuls release those expT rows), the second
                # half right after the chunk evict — so Vector's in-order
                # queue never parks an evict behind a tail reciprocal that
                # is still waiting on the GpSimd all-reduce.
                ah, ac = av_pair
                aexp = expT_of[av_pair]
                op = opp.tile([128, CH], fp32, tag="op", name="op")
                tail_head = None
                for tt2 in range(NT // 2):
                    if tt2 < nslots:
                        emit_score_slot()
                    for i in range(2):
                        tt = 2 * tt2 + i
                        nc.tensor.matmul(
                            op[:], vc[:, tt, ah * 128:(ah + 1) * 128],
                            aexp[:, tt, :], start=(tt == 0),
                            stop=(tt == NT - 1))
                    chunk_mms(ch, 2)
                    if tt2 == 3 and pending:
                        tail_head = emit_tail_head(pending.pop(0))
                    if tt2 == 4:
                        nc.vector.tensor_add(aexp[:, 0:4, :],
                                             aexp[:, 0:4, :],
                                             aexp[:, 4:8, :])
                    if tt2 == 5:
                        nc.vector.tensor_add(aexp[:, 0:2, :],
                                             aexp[:, 0:2, :],
                                             aexp[:, 2:4, :])
                    if tt2 == 6:
                        nc.vector.tensor_add(aexp[:, 0:1, :],
                                             aexp[:, 0:1, :],
                                             aexp[:, 1:2, :])
                finish_chunk(ch)
                if tail_head is not None:
                    emit_tail_mul(tail_head)
                pexp = expT_of.pop(av_pair)
                nc.vector.tensor_add(pexp[:, 8:12, :], pexp[:, 8:12, :],
                                     pexp[:, 12:16, :])
                nc.vector.tensor_add(pexp[:, 8:10, :], pexp[:, 8:10, :],
                                     pexp[:, 10:12, :])
                nc.vector.tensor_add(pexp[:, 8:9, :], pexp[:, 8:9, :],
                                     pexp[:, 9:10, :])
                acc = accp.tile([128, CH], fp16, tag="acc", name="acc")
                nc.vector.tensor_add(acc[:], pexp[:, 0, :], pexp[:, 8, :])
                pending.append((av_pair, acc, op))

            chunk_queue = [(h, nn) for h in range(NH) for nn in range(NOC)]
            pending = []
            budgets = [6, 6, 6] + [8] * 11 + [6, 0]
            for j, cur in enumerate(atts):
                ch = None
                # j<15 keeps the remaining head-2 chunks for the drain,
                # where their matmuls cover the final denominator chains.
                if (chunk_queue and j < 15
                        and j >= 4 * (chunk_queue[0][0] + 1) + 1):
                    ch = start_chunk(*chunk_queue.pop(0))
                att_step(cur, budgets[j], pending, ch,
                         last=(j == len(atts) - 1))
                # Wo chunk slices stream in during the first attention
                # steps (x traffic is over; gpsimd queue is idle)
                if j < NOC:
                    load_wo(j)

            # ===== drain: last tail + remaining O-proj chunks =====
            # The final pair's denominator chain is emitted first (highest
            # priority), then the remaining chunks; (2,2)/(2,3) have no
            # dependence on the final tails, so the scheduler interleaves
            # their matmuls under the chain's latency, and the head-3
            # chunks start once the final normalize lands in oc.
            emit_tail(pending.pop(0))
            alt = 0
            while chunk_queue:
                last_chunk = len(chunk_queue) == 1
                # alternate PSUM rings so back-to-back drain chunks never
                # wait on each other's evict
                if alt % 2 == 0:
                    ch = start_chunk(*chunk_queue.pop(0))
                else:
                    ch = start_chunk(*chunk_queue.pop(0), pool=opp,
                                     tag="op")
                alt += 1
                if last_chunk:
                    finish_chunk_split(ch)
                else:
                    chunk_mms(ch, NK)
                    finish_chunk(ch)

    nc.compile()
    return nc


def _tile_x(xt, NK2, NPC):
    # (E, S) fp16 -> [k_pair, s_chunk, 128, 2*512] contiguous (2KB rows)
    return np.ascontiguousarray(
        xt.reshape(NK2, 2, 128, NPC, 512).transpose(0, 3, 2, 1, 4)
        .reshape(NK2, NPC, 128, 1024))


def _tile_w(wT, NK2, HDc):
    # (E, HDc) fp16 -> [k_pair, 128, 2, HDc] (2KB rows)
    return np.ascontiguousarray(
        wT.reshape(NK2, 2, 128, HDc).transpose(0, 2, 1, 3))


def shard_inputs(cfg: Cfg, query, key, value, Wq, bq, Wk, bk, Wv, bv, Wo, bo):
    """Build per-core in_maps from full inputs."""
    f = np.float32
    h16 = np.float16
    query, key, value = (np.asarray(a, f) for a in (query, key, value))
    Wq, Wk, Wv, Wo = (np.asarray(a, f) for a in (Wq, Wk, Wv, Wo))
    bq, bk, bv, bo = (np.asarray(a, f) for a in (bq, bk, bv, bo))
    NH, HDc, NK2, NPC = cfg.NH, cfg.HDc, cfg.NK2, cfg.NPC
    NOC = cfg.NOC
    # Wo^T -> [nn, k_pair, 128, 2, 512] (2KB rows)
    wo_t = np.ascontiguousarray(
        Wo.T.astype(h16).reshape(NK2, 2, 128, NOC, 512)
        .transpose(3, 0, 2, 1, 4))
    _ONES = np.ones((128, 1), np.float32)
    xq_t = [_tile_x(query[n].T.astype(h16), NK2, NPC) for n in range(N_BATCH)]
    xk_t = [_tile_x(key[n].T.astype(h16), NK2, NPC) for n in range(N_BATCH)]
    xv_t = [_tile_x(value[n].T.astype(h16), NK2, NPC) for n in range(N_BATCH)]
    in_maps = []
    cores_per_batch = N_CORES // N_BATCH
    for c in range(N_CORES):
        n = c // cores_per_batch
        hs = (c % cores_per_batch) * HDc
        sl = slice(hs, hs + HDc)
        in_maps.append({
            "xq": xq_t[n],
            "xk": xk_t[n],
            "xv": xv_t[n],
            "wq": _tile_w(np.ascontiguousarray(Wq[sl].T).astype(h16),
                          NK2, HDc),
            "wk": _tile_w(np.ascontiguousarray(Wk[sl].T).astype(h16),
                          NK2, HDc),
            "wv": _tile_w(np.ascontiguousarray(Wv[sl].T).astype(h16),
                          NK2, HDc),
            "wo": wo_t,
            "bq": np.ascontiguousarray(bq[sl].reshape(NH, 128).T),
            "bk": np.ascontiguousarray(bk[sl].reshape(NH, 128).T),
            "bv": np.ascontiguousarray(bv[sl].reshape(1, HDc)).astype(h16),
            "ones": _ONES.astype(h16),
        })
    return in_maps


def gather_outputs(cfg: Cfg, results, bo):
    """results: list of per-core {'out': (NH*128, E)} -> full (N, S, E).

    bo is a per-column additive constant on the final output, applied
    here instead of on-device (exact)."""
    E = cfg.E
    full = np.empty((N_BATCH, SEQ, E), np.float32)
    cores_per_batch = N_CORES // N_BATCH
    rows = cfg.NH * 128
    for c in range(N_CORES):
        n = c // cores_per_batch
        r0 = (c % cores_per_batch) * rows
        full[n, r0:r0 + rows, :] = results[c]["out"]
    full += np.asarray(bo, np.float32)[None, None, :]
    return full


_CACHE = {}


def kernel(**inputs) -> np.ndarray:
    from concourse.bass_utils import run_bass_kernel_spmd
    cfg = Cfg()
    if "nc" not in _CACHE:
        _CACHE["nc"] = build_program(cfg)
    nc = _CACHE["nc"]
    in_maps = shard_inputs(cfg, **inputs)
    res = run_bass_kernel_spmd(nc, in_maps, core_ids=list(range(N_CORES)))
    return gather_outputs(cfg, res.results, inputs["bo"])


# revision 114
# speedup vs baseline: 1.0217x; 1.0217x over previous
# Multi-head attention (N=2, S=2048, E=2048, H=16, Dk=128) on 8 NeuronCores.
#
# Sharding: 2 batches x 16 heads = 32 (n,h) pairs -> core c owns batch c//4,
# heads (c%4)*4 .. +4. The reference reshapes (N,H,S,Dk)->(N,S,H*Dk) without
# a head transpose, so rows [h*128,(h+1)*128) of the pre-projection matrix X
# (and hence of the final output) depend on head h only: each core computes
# 512 disjoint output rows and the host concatenates. No collectives.
#
# v7 (over v6): the softmax add-tree is split -- first half in-place across
# attention slots 4-6 as attnV releases those expT rows, second half right
# after the chunk evict -- so Vector's in-order queue never parks an evict
# behind a tail reciprocal; the tail's normalize-mul is emitted after the
# evict for the same reason. V-phase x tiles get a dedicated prefetch ring
# (vin) and wpool bufs=3 so wv's DMAs are not gated on Q-proj's last
# LDWEIGHTS; weight/x descriptors are spread across scalar/gpsimd/sync by
# phase. The last Q/K s-chunk's evicts alternate Scalar/Vector because the
# freed pa PSUM banks become the V/score rings. bo is folded in on the host
# (per-column additive constant), removing its 2.2us gpsimd broadcast; the
# drain reserves two head-2 chunks whose matmuls cover the final
# denominator chain, and the last O-proj chunk computes + evicts in column
# halves so the output DMA overlaps its matmuls.
import numpy as np

D_MODEL = 2048
NHEAD = 16
DK = 128
N_BATCH = 2
SEQ = 2048
N_CORES = 8
HEADS_PER_CORE = 4


class Cfg:
    def __init__(self, S=SEQ, E=D_MODEL, NH=HEADS_PER_CORE, CH=512):
        assert S % 128 == 0 and E % 128 == 0
        self.S = S          # sequence length
        self.E = E          # model dim (contraction for projections)
        self.NH = NH        # heads per core
        self.CH = CH        # s-chunk width for attention phase
        self.NK = E // 128  # contraction tiles for projections / O-proj
        self.NK2 = self.NK // 2  # k-pair tiles (DMA granularity)
        self.NT = S // 128  # t tiles (attention contraction)
        self.HDc = NH * DK  # head dims per core
        self.RPH = (S * DK) // E  # output rows per head (=128 at full size)
        assert self.RPH == 128, "O-proj layout assumes 128 rows per head"
        self.NCH = S // CH  # number of s-chunks
        assert S % CH == 0 and CH == 512
        self.PCH = 512      # projection / O-proj free-dim chunk
        self.NPC = S // self.PCH   # projection s-chunks
        self.NOC = E // self.PCH   # O-proj output chunks


def build_program(cfg: Cfg):
    import concourse.bass as bass
    import concourse.tile as tile
    from concourse import bacc, mybir
    from contextlib import ExitStack

    fp32 = mybir.dt.float32
    fp16 = mybir.dt.float16
    AF = mybir.ActivationFunctionType

    S, E, NH, CH = cfg.S, cfg.E, cfg.NH, cfg.CH
    NK, NK2, NT, HDc = cfg.NK, cfg.NK2, cfg.NT, cfg.HDc
    PCH, NPC, NOC, NCH = cfg.PCH, cfg.NPC, cfg.NOC, cfg.NCH
    inv_sqrt_dk = 1.0 / float(np.sqrt(DK))

    nc = bacc.Bacc("TRN2", target_bir_lowering=False, debug=False,
                   num_devices=N_CORES)

    # DRAM I/O (per-core values supplied via in_maps). x inputs are
    # host-pretiled fp16: [k_pair, s_chunk, partition, 2*512] so every DMA
    # reads contiguous 2KB partition rows.
    xq = nc.dram_tensor("xq", [NK2, NPC, 128, 1024], fp16,
                        kind="ExternalInput").ap()
    xk = nc.dram_tensor("xk", [NK2, NPC, 128, 1024], fp16,
                        kind="ExternalInput").ap()
    xv = nc.dram_tensor("xv", [NK2, NPC, 128, 1024], fp16,
                        kind="ExternalInput").ap()
    wq = nc.dram_tensor("wq", [NK2, 128, 2, HDc], fp16,
                        kind="ExternalInput").ap()
    wk = nc.dram_tensor("wk", [NK2, 128, 2, HDc], fp16,
                        kind="ExternalInput").ap()
    wv = nc.dram_tensor("wv", [NK2, 128, 2, HDc], fp16,
                        kind="ExternalInput").ap()
    wo = nc.dram_tensor("wo", [NOC, NK2, 128, 2, 512], fp16,
                        kind="ExternalInput").ap()
    bq = nc.dram_tensor("bq", [128, NH], fp32, kind="ExternalInput").ap()
    bk = nc.dram_tensor("bk", [128, NH], fp32, kind="ExternalInput").ap()
    bv = nc.dram_tensor("bv", [1, HDc], fp16, kind="ExternalInput").ap()
    out = nc.dram_tensor("out", [NH * 128, E], fp16, kind="ExternalOutput").ap()

    with tile.TileContext(nc) as tc, ExitStack() as ctx:
        consts = ctx.enter_context(tc.tile_pool(name="consts", bufs=1))
        ones_sb = consts.tile([128, 1], fp16)
        ones_col = ones_sb[:, :1]
        bq_sb = consts.tile([128, NH], fp32)
        bk_sb = consts.tile([128, NH], fp32)
        bv_sb = consts.tile([1, HDc], fp16)
        bv_bc = consts.tile([128, HDc], fp16)   # bv broadcast along t

        persist = ctx.enter_context(tc.tile_pool(name="persist", bufs=1))
        qc = persist.tile([128, NH, S], fp16)      # qT_c: [d, h, s]
        kc = persist.tile([128, NH, S], fp16)      # kT_c: [d, h, s]
        vc = persist.tile([128, NT, HDc], fp16)    # v_c: [t_p, t_t, h*128+d]
        oc = persist.tile([128, NH, S], fp16)      # attn out: [d, h, s]

        # SBUF pools that span phase A tail + phase B (pre-scored pairs).
        expp = ctx.enter_context(tc.tile_pool(name="expp", bufs=4))
        accp = ctx.enter_context(tc.tile_pool(name="accp", bufs=1))
        bsc = ctx.enter_context(tc.tile_pool(name="bsc", bufs=2))
        expT_of = {}

        # ============== Phase A: q/k/v projections ==============
        from contextlib import ExitStack as _ES
        a_ctx = _ES()
        # bufs=3: wq/wk/wv each get fresh slots, so wv's DMAs are not
        # gated on Q-proj's last LDWEIGHTS releasing wq's tiles (that
        # gating was a ~1.9us PE stall at the K->V transition)
        wpool = a_ctx.enter_context(tc.tile_pool(name="wpool", bufs=3))
        xin = a_ctx.enter_context(tc.tile_pool(name="xin", bufs=6))
        # dedicated prefetch ring for V x tiles: their DMAs hoist into the
        # K-projection window instead of waiting for xin slots, so the PE
        # has V matmuls available the moment K-proj ends.
        vin = a_ctx.enter_context(tc.tile_pool(name="vin", bufs=4))
        with tc.tile_pool(name="pa_psum", bufs=2, space="PSUM") as pa:

            def load_w(w_dram, engs, gate=None, first=False):
                # 4 sub-tiles of 2 k-pairs each so the first matmuls only
                # wait on the first pair of weight DMAs, not all eight.
                # `gate` writes one element into each destination tile so
                # the DMAs cannot be scheduled before the gate source is
                # produced (keeps them off the startup DMA window).
                # `first`: the leading k-pair is split k-granular so the
                # program's first LDWEIGHTS waits on a 128KB DMA, not 256.
                parts = []
                for i in range(4):
                    p = wpool.tile([128, 4, HDc], fp16, tag=f"w{i}",
                                   name=f"w{i}")
                    if gate is not None:
                        nc.gpsimd.tensor_copy(p[0:1, 0, 0:1], gate)
                    if first and i == 0:
                        engs[0].dma_start(p[:, 0:1, :], w_dram[0][:, 0:1, :])
                        engs[0].dma_start(p[:, 1:2, :], w_dram[0][:, 1:2, :])
                        engs[0].dma_start(p[:, 2:4, :], w_dram[1])
                    else:
                        for j in range(2):
                            engs[i].dma_start(p[:, 2 * j:2 * j + 2, :],
                                              w_dram[2 * i + j])
                    parts.append(p)
                return parts

            def proj_qk(w_sb, x_dram, bias_sb, dst, eng, eng2=None):
                # dst[:, m, s*] = W_c @ x^T  (hd x S), bias fused in evict
                for s in range(NPC):
                    ps = [pa.tile([128, PCH], fp32, tag=f"pa{m}",
                                  name=f"pa{m}") for m in range(NH)]
                    for k2 in range(NK2):
                        xtile = xin.tile([128, 1024], fp16, tag="xin")
                        if eng2 == "first" and s == 0 and k2 == 0:
                            # split the program's first x tile: matmuls
                            # k=0,1 only need the kk=0 half
                            eng.dma_start(xtile[:, 0:512],
                                          x_dram[0, 0][:, 0:512])
                            eng.dma_start(xtile[:, 512:1024],
                                          x_dram[0, 0][:, 512:1024])
                        else:
                            eng.dma_start(xtile[:], x_dram[k2, s])
                        for kk in range(2):
                            k = 2 * k2 + kk
                            xs = xtile[:, kk * 512:(kk + 1) * 512]
                            for m in range(NH):
                                nc.tensor.matmul(
                                    ps[m][:],
                                    w_sb[k // 4][:, k % 4,
                                                 m * 128:(m + 1) * 128],
                                    xs, start=(k == 0), stop=(k == NK - 1))
                    for m in range(NH):
                        # last s-chunk: alternate evict engines -- the
                        # freed pa banks become the V/score PSUM rings, so
                        # the serialized 2.4us Scalar evict train directly
                        # delays the next phase's first matmuls
                        if s == NPC - 1 and m % 2 == 1:
                            nc.vector.tensor_scalar_add(
                                dst[:, m, s * PCH:(s + 1) * PCH],
                                ps[m][:], bias_sb[:, m:m + 1])
                        else:
                            nc.scalar.activation(
                                dst[:, m, s * PCH:(s + 1) * PCH],
                                ps[m][:], AF.Identity,
                                bias=bias_sb[:, m:m + 1])

            # wq's later sub-tiles stream on gpsimd (idle at startup) so
            # the scalar descriptor queue only carries what gates the
            # first matmuls; wk streams on gpsimd behind a qc gate that
            # keeps it out of wq's startup DMA window; wv goes back on
            # scalar BEFORE K-proj is emitted, so its descriptors run
            # between the Q and K evict trains and the tiles are resident
            # well before the V matmuls start.
            def pa_ring_skip():
                # advance the pa PSUM ring one slot-set so K-proj's final
                # s-chunk lands on the other half of the ring: the banks
                # pav inherits are then free ~14us before V-proj starts
                # instead of being read by K's last evicts.
                for m in range(NH):
                    pa.tile([128, PCH], fp32, tag=f"pa{m}",
                            name=f"pa{m}_skip")

            wq_sb = load_w(wq, [nc.scalar, nc.scalar, nc.gpsimd, nc.gpsimd])
            # ones column via memset on the startup-idle Vector engine:
            # one less DMA descriptor in the contended startup window
            nc.vector.memset(ones_sb[:], 1.0)
            nc.scalar.dma_start(bq_sb[:], bq)
            nc.scalar.dma_start(bk_sb[:], bk)
            nc.scalar.dma_start(bv_sb[:], bv)
            proj_qk(wq_sb, xq, bq_sb, qc, nc.sync)
            # gate on Q's SECOND s-chunk evict (~39us): wk's 2MB stream
            # then stays out of the most contended stretch of the Q-phase
            # x/w traffic (8 cores all pulling weights at once); its real
            # deadline is K-proj start at ~67us.
            wk_sb = load_w(wk, [nc.gpsimd] * 4, gate=qc[0:1, 0, 512:513])
            wv_sb = load_w(wv, [nc.scalar] * 4)
            pa_ring_skip()
            proj_qk(wk_sb, xk, bk_sb, kc, nc.gpsimd)

            # bv broadcast tile (GpSimd partition-broadcast)
            nc.gpsimd.partition_broadcast(bv_bc[:], bv_sb[:])

        def emit_scores(cur):
            # scores + exp for one (h, c) pair; expT kept for the attnV
            # step that consumes it later.
            h, c = cur
            cs = slice(c * CH, (c + 1) * CH)
            expT = expp.tile([128, NT, CH], fp16, tag="expT",
                             name=f"expT_{h}_{c}")
            for tt2 in range(NT // 2):
                st = stp.tile([128, 2, 512], fp32, tag="st", name="st")
                for i in range(2):
                    tt = 2 * tt2 + i
                    nc.tensor.matmul(
                        st[:, i, :], kc[:, h, tt * 128:(tt + 1) * 128],
                        qc[:, h, cs], start=True, stop=True)
                nc.scalar.activation(expT[:, 2 * tt2:2 * tt2 + 2, :],
                                     st[:], AF.Exp, scale=inv_sqrt_dk)
            expT_of[cur] = expT

        atts = [(h, c) for h in range(NH) for c in range(NCH)]

        # pa closed: scores PSUM ring opens (spans pre-scores + phase B)
        stp = ctx.enter_context(
            tc.tile_pool(name="st_psum", bufs=2, space="PSUM"))

        with tc.tile_pool(name="pav_psum", bufs=1, space="PSUM") as pav:

            def proj_v_group(tc4):
                # 4 t-tiles of v: stationary = x tile slices, moving = w
                ps = [pav.tile([128, HDc], fp32, tag=f"pav{j}",
                               name=f"pav{j}") for j in range(4)]
                for k2 in range(NK2):
                    xtile = vin.tile([128, 1024], fp16, tag="vin")
                    # group 0 entirely on sync (gpsimd is still draining
                    # K descriptors when V-proj starts); mid groups' odd
                    # halves go on scalar, which is idle after K's evicts
                    if k2 % 2 == 0 or tc4 == 0:
                        eng = nc.sync
                    elif tc4 < 3:
                        eng = nc.scalar
                    else:
                        eng = nc.gpsimd
                    eng.dma_start(xtile[:], xv[k2, tc4])
                    for kk in range(2):
                        k = 2 * k2 + kk
                        for j in range(4):
                            xs = xtile[:, kk * 512 + j * 128:
                                       kk * 512 + (j + 1) * 128]
                            nc.tensor.matmul(
                                ps[j][:], xs, wv_sb[k // 4][:, k % 4, :],
                                start=(k == 0), stop=(k == NK - 1))
                for j in range(4):
                    nc.vector.tensor_add(vc[:, tc4 * 4 + j, :], ps[j][:],
                                         bv_bc[:])

            # Pre-score two pairs between the V groups (V first, so the
            # PE never waits on the K evicts): Scalar banks a 2-pair exp
            # lead during the v-projection; the lead absorbs Scalar's
            # per-step deficit for the whole attention phase.
            proj_v_group(0)
            emit_scores(atts[0])
            proj_v_group(1)
            proj_v_group(2)
            emit_scores(atts[1])
            for g in range(3, NT // 4):
                proj_v_group(g)

        a_ctx.close()

        # ============== Phase B: attention ==============
        wop = ctx.enter_context(tc.tile_pool(name="wop", bufs=4))
        wo_tiles = {}

        def load_wo(nn):
            # tiny vc-sourced copies gate each DMA so the scheduler cannot
            # hoist these dep-free loads into the phase-A DMA window; the
            # DMA issues go on sync (idle in early B) so they never delay
            # the per-tail partition ops on gpsimd.
            wo_t = wop.tile([128, NK, 512], fp16, tag="wo", name=f"wo{nn}")
            for k2 in range(NK2):
                nc.gpsimd.tensor_copy(wo_t[:, 2 * k2, 0:1],
                                      vc[:, NT - 1, HDc - 1:HDc])
                nc.sync.dma_start(wo_t[:, 2 * k2:2 * k2 + 2, :],
                                  wo[nn, k2])
            wo_tiles[nn] = wo_t

        with tc.tile_pool(name="op_psum", bufs=2, space="PSUM") as opp, \
             tc.tile_pool(name="pso_psum", bufs=1, space="PSUM") as psop, \
             tc.tile_pool(name="dn_psum", bufs=1, space="PSUM") as dnpp:

            def emit_tail_head(t):
                # denominator part 2a: ones-matmul column sum, fast
                # reciprocal, GpSimd partition-broadcast. The normalize
                # multiply is emitted separately AFTER the chunk evict so
                # a slow broadcast can never park the evict behind it on
                # Vector's in-order queue. (A GpSimd partition_all_reduce
                # variant was tried and reverted: its real 4-5us latency
                # is invisible to the static scheduler's cost model.)
                (ph, pc), acc, op = t
                dn = dnpp.tile([1, CH], fp32, tag="dn", name="dn")
                nc.tensor.matmul(dn[:], ones_col, acc[:],
                                 start=True, stop=True)
                rsc1 = bsc.tile([1, CH], fp32, tag="rsc1", name="rsc1")
                nc.vector.reciprocal_approx_fast(rsc1[:], dn[:])
                rsc = bsc.tile([128, CH], fp32, tag="rsc", name="rsc")
                nc.gpsimd.partition_broadcast(rsc[:], rsc1[:])
                return ((ph, pc), op, rsc)

            def emit_tail_mul(th):
                (ph, pc), op, rsc = th
                nc.vector.tensor_mul(oc[:, ph, pc * CH:(pc + 1) * CH],
                                     op[:], rsc[:])

            def emit_tail(t):
                emit_tail_mul(emit_tail_head(t))

            osb = ctx.enter_context(tc.tile_pool(name="osb", bufs=2))

            def start_chunk(h, nn, pool=None, tag="pso"):
                pool = pool or psop
                return {"h": h, "nn": nn, "k": 0,
                        "ps": pool.tile([128, PCH], fp32, tag=tag,
                                        name="pso"),
                        "ocv": oc[:, h, :].rearrange("p (j i) -> p i j",
                                                     i=NK),
                        "wo": wo_tiles[nn]}

            def chunk_mms(ch, n):
                for _ in range(n):
                    if ch is None or ch["k"] >= NK:
                        return
                    k = ch["k"]
                    nc.tensor.matmul(ch["ps"][:], ch["ocv"][:, k, :],
                                     ch["wo"][:, k, :],
                                     start=(k == 0), stop=(k == NK - 1))
                    ch["k"] += 1

            def finish_chunk(ch):
                # bo is folded in on the host after gather (exact: it is a
                # per-column additive constant), so the evict is a copy.
                if ch is None:
                    return
                chunk_mms(ch, NK - ch["k"])
                ns = slice(ch["nn"] * PCH, (ch["nn"] + 1) * PCH)
                ot = osb.tile([128, PCH], fp16, tag="osb")
                nc.vector.tensor_copy(ot[:], ch["ps"][:])
                # out DMAs issue on gpsimd: keeps that engine active every
                # step (its broadcasts run ~2x slower after idle spells)
                # and relieves sync, the busiest descriptor queue
                nc.gpsimd.dma_start(
                    out[ch["h"] * 128:(ch["h"] + 1) * 128, ns], ot[:])

            def finish_chunk_split(ch):
                # last chunk: compute + evict in two column halves so the
                # first half's bias-add and output DMA overlap the second
                # half's matmuls (shrinks the post-PE tail).
                ns0 = ch["nn"] * PCH
                ot = osb.tile([128, PCH], fp16, tag="osb")
                for half in range(2):
                    hs = slice(half * 256, (half + 1) * 256)
                    for k in range(NK):
                        nc.tensor.matmul(
                            ch["ps"][:, hs], ch["ocv"][:, k, :],
                            ch["wo"][:, k, hs],
                            start=(k == 0), stop=(k == NK - 1))
                    nc.vector.tensor_copy(ot[:, hs], ch["ps"][:, hs])
                    nc.sync.dma_start(
                        out[ch["h"] * 128:(ch["h"] + 1) * 128,
                            ns0 + half * 256:ns0 + (half + 1) * 256],
                        ot[:, hs])

            score_q = [(p, t) for p in range(2, 16) for t in range(8)]
            score_state = {"i": 0, "open": {}}

            def emit_score_slot():
                # one st-ring slot (2 tt) of the global score stream; the
                # per-step budget is lighter in the Scalar-bound early
                # steps and the deficit lands on steps 13-14 where Scalar
                # is otherwise idle
                if score_state["i"] >= len(score_q):
                    return
                p, tt2 = score_q[score_state["i"]]
                score_state["i"] += 1
                h, c = atts[p]
                if tt2 == 0:
                    score_state["open"][p] = expp.tile(
                        [128, NT, CH], fp16, tag="expT",
                        name=f"expT_{h}_{c}")
                expT = score_state["open"][p]
                cs = slice(c * CH, (c + 1) * CH)
                st = stp.tile([128, 2, 512], fp32, tag="st", name="st")
                for i in range(2):
                    tt = 2 * tt2 + i
                    nc.tensor.matmul(
                        st[:, i, :], kc[:, h, tt * 128:(tt + 1) * 128],
                        qc[:, h, cs], start=True, stop=True)
                nc.scalar.activation(expT[:, 2 * tt2:2 * tt2 + 2, :],
                                     st[:], AF.Exp, scale=inv_sqrt_dk)
                if tt2 == 7:
                    expT_of[atts[p]] = score_state["open"].pop(p)

            def att_step(av_pair, nslots, pending, ch, last=False):
                # Interleave scores(j+2) tt-pairs with attnV(j) tt-pairs
                # plus two O-proj matmuls per slot: the filler keeps each
                # st-ring slot interval above the Scalar exp latency so the
                # score stream is never exp-paced. The denominator add-tree
                # runs split: the first half in-place across slots 4-6 (as
                # the attnV matmuls release those expT rows), the second
                # half right after the chunk evict — so Vector's in-order
                # queue never parks an evict behind a tail reciprocal that
                # is still waiting on the GpSimd all-reduce.
                ah, ac = av_pair
                aexp = expT_of[av_pair]
                op = opp.tile([128, CH], fp32, tag="op", name="op")
                tail_head = None
                for tt2 in range(NT // 2):
                    if tt2 < nslots:
                        emit_score_slot()
                    for i in range(2):
                        tt = 2 * tt2 + i
                        nc.tensor.matmul(
                            op[:], vc[:, tt, ah * 128:(ah + 1) * 128],
                            aexp[:, tt, :], start=(tt == 0),
                            stop=(tt == NT - 1))
                    chunk_mms(ch, 2)
                    if tt2 == 3 and pending:
                        tail_head = emit_tail_head(pending.pop(0))
                    if tt2 == 4:
                        nc.vector.tensor_add(aexp[:, 0:4, :],
                                             aexp[:, 0:4, :],
                                             aexp[:, 4:8, :])
                    if tt2 == 5:
                        nc.vector.tensor_add(aexp[:, 0:2, :],
                                             aexp[:, 0:2, :],
                                             aexp[:, 2:4, :])
                    if tt2 == 6:
                        nc.vector.tensor_add(aexp[:, 0:1, :],
                                             aexp[:, 0:1, :],
                                             aexp[:, 1:2, :])
                finish_chunk(ch)
                if tail_head is not None:
                    emit_tail_mul(tail_head)
                pexp = expT_of.pop(av_pair)
                nc.vector.tensor_add(pexp[:, 8:12, :], pexp[:, 8:12, :],
                                     pexp[:, 12:16, :])
                nc.vector.tensor_add(pexp[:, 8:10, :], pexp[:, 8:10, :],
                                     pexp[:, 10:12, :])
                nc.vector.tensor_add(pexp[:, 8:9, :], pexp[:, 8:9, :],
                                     pexp[:, 9:10, :])
                acc = accp.tile([128, CH], fp16, tag="acc", name="acc")
                nc.vector.tensor_add(acc[:], pexp[:, 0, :], pexp[:, 8, :])
                pending.append((av_pair, acc, op))

            chunk_queue = [(h, nn) for h in range(NH) for nn in range(NOC)]
            pending = []
            budgets = [6, 6, 6] + [8] * 11 + [6, 0]
            for j, cur in enumerate(atts):
                ch = None
                # j<15 keeps the remaining head-2 chunks for the drain,
                # where their matmuls cover the final denominator chains.
                if (chunk_queue and j < 15
                        and j >= 4 * (chunk_queue[0][0] + 1) + 1):
                    ch = start_chunk(*chunk_queue.pop(0))
                att_step(cur, budgets[j], pending, ch,
                         last=(j == len(atts) - 1))
                # Wo chunk slices stream in during the first attention
                # steps (x traffic is over; gpsimd queue is idle)
                if j < NOC:
                    load_wo(j)

            # ===== drain: last tail + remaining O-proj chunks =====
            # The final pair's denominator chain is emitted first (highest
            # priority), then the remaining chunks; (2,2)/(2,3) have no
            # dependence on the final tails, so the scheduler interleaves
            # their matmuls under the chain's latency, and the head-3
            # chunks start once the final normalize lands in oc.
            emit_tail(pending.pop(0))
            alt = 0
            while chunk_queue:
                last_chunk = len(chunk_queue) == 1
                # alternate PSUM rings so back-to-back drain chunks never
                # wait on each other's evict
                if alt % 2 == 0:
                    ch = start_chunk(*chunk_queue.pop(0))
                else:
                    ch = start_chunk(*chunk_queue.pop(0), pool=opp,
                                     tag="op")
                alt += 1
                if last_chunk:
                    finish_chunk_split(ch)
                else:
                    chunk_mms(ch, NK)
                    finish_chunk(ch)

    nc.compile()
    return nc


def _tile_x(xt, NK2, NPC):
    # (E, S) fp16 -> [k_pair, s_chunk, 128, 2*512] contiguous (2KB rows)
    return np.ascontiguousarray(
        xt.reshape(NK2, 2, 128, NPC, 512).transpose(0, 3, 2, 1, 4)
        .reshape(NK2, NPC, 128, 1024))


def _tile_w(wT, NK2, HDc):
    # (E, HDc) fp16 -> [k_pair, 128, 2, HDc] (2KB rows)
    return np.ascontiguousarray(
        wT.reshape(NK2, 2, 128, HDc).transpose(0, 2, 1, 3))


def shard_inputs(cfg: Cfg, query, key, value, Wq, bq, Wk, bk, Wv, bv, Wo, bo):
    """Build per-core in_maps from full inputs."""
    f = np.float32
    h16 = np.float16
    query, key, value = (np.asarray(a, f) for a in (query, key, value))
    Wq, Wk, Wv, Wo = (np.asarray(a, f) for a in (Wq, Wk, Wv, Wo))
    bq, bk, bv, bo = (np.asarray(a, f) for a in (bq, bk, bv, bo))
    NH, HDc, NK2, NPC = cfg.NH, cfg.HDc, cfg.NK2, cfg.NPC
    NOC = cfg.NOC
    # Wo^T -> [nn, k_pair, 128, 2, 512] (2KB rows)
    wo_t = np.ascontiguousarray(
        Wo.T.astype(h16).reshape(NK2, 2, 128, NOC, 512)
        .transpose(3, 0, 2, 1, 4))
    xq_t = [_tile_x(query[n].T.astype(h16), NK2, NPC) for n in range(N_BATCH)]
    xk_t = [_tile_x(key[n].T.astype(h16), NK2, NPC) for n in range(N_BATCH)]
    xv_t = [_tile_x(value[n].T.astype(h16), NK2, NPC) for n in range(N_BATCH)]
    in_maps = []
    cores_per_batch = N_CORES // N_BATCH
    for c in range(N_CORES):
        n = c // cores_per_batch
        hs = (c % cores_per_batch) * HDc
        sl = slice(hs, hs + HDc)
        in_maps.append({
            "xq": xq_t[n],
            "xk": xk_t[n],
            "xv": xv_t[n],
            "wq": _tile_w(np.ascontiguousarray(Wq[sl].T).astype(h16),
                          NK2, HDc),
            "wk": _tile_w(np.ascontiguousarray(Wk[sl].T).astype(h16),
                          NK2, HDc),
            "wv": _tile_w(np.ascontiguousarray(Wv[sl].T).astype(h16),
                          NK2, HDc),
            "wo": wo_t,
            "bq": np.ascontiguousarray(bq[sl].reshape(NH, 128).T),
            "bk": np.ascontiguousarray(bk[sl].reshape(NH, 128).T),
            "bv": np.ascontiguousarray(bv[sl].reshape(1, HDc)).astype(h16),
        })
    return in_maps


def gather_outputs(cfg: Cfg, results, bo):
    """results: list of per-core {'out': (NH*128, E)} -> full (N, S, E).

    bo is a per-column additive constant on the final output, applied
    here instead of on-device (exact)."""
    E = cfg.E
    full = np.empty((N_BATCH, SEQ, E), np.float32)
    cores_per_batch = N_CORES // N_BATCH
    rows = cfg.NH * 128
    for c in range(N_CORES):
        n = c // cores_per_batch
        r0 = (c % cores_per_batch) * rows
        full[n, r0:r0 + rows, :] = results[c]["out"]
    full += np.asarray(bo, np.float32)[None, None, :]
    return full


_CACHE = {}


def kernel(**inputs) -> np.ndarray:
    from concourse.bass_utils import run_bass_kernel_spmd
    cfg = Cfg()
    if "nc" not in _CACHE:
        _CACHE["nc"] = build_program(cfg)
    nc = _CACHE["nc"]
    in_maps = shard_inputs(cfg, **inputs)
    res = run_bass_kernel_spmd(nc, in_maps, core_ids=list(range(N_CORES)))
    return gather_outputs(cfg, res.results, inputs["bo"])
